# revision 22
# baseline (speedup 1.0000x reference)
"""Trainium2 Bass kernel for MemoryAsContextTransformer segmented attention.

Reference computation (per full input):
  h   = rmsnorm(x, gamma)                      [B=2, S=4096, D=1024]
  qkv = h @ w_qkv                              heads=16, dh=64, seg=512, pm=16
  per (batch, segment, head): block-causal attention with 16 persistent
  memory tokens prepended to k/v, softmax, out = attn @ v
  out @ w_out                                  [2, 4096, 1024]

Sharding: data-parallel over the 16 (batch, segment) units; 2 contiguous
segments (1024 tokens) per core, full weights broadcast to all 8 cores.

Kernel design (all matmul operands bf16, f32 PSUM accumulation):
  x arrives host-pre-transposed as xT [d, t]; rmsnorm computed via
  PE partition-reduction (ones-column matmul of x^2) + Act ln/exp for
  rsqrt + PE one-hot broadcast; h = xT * rs on DVE. No PE transposes.
  Projections: stationary weight tiles resident in SBUF, 8-matmul
  accumulation chains at N=512.
  Attention uses a shifted-block scheme: k/v get the 16 persistent-memory
  rows prepended, blocks of 128 k-rows start at -16 so the pm rows ride in
  block 0; per (head, seg): 5 sim matmuls into one 3-bank PSUM tile, ONE
  exp over it (Act), 5 causal-mask muls (GpSimd), 5 PV matmuls with an
  extra ones column for the softmax denominators.
  Normalization: denominator rows gathered per segment via tiny SBUF DMAs,
  reciprocal (DVE), one-hot head-broadcast matmul, DVE muls.
  Out-projection chains per segment; seg0's runs interleaved into seg1's
  attention, and seg1's projections interleave into seg0's attention, so
  the PE queue always has independent work while Act runs the exps.
"""

import sys

sys.path.insert(0, "/opt/trn_rl_repo")

from contextlib import ExitStack

import numpy as np
import ml_dtypes

import concourse.bass as bass
import concourse.mybir as mybir
import concourse.tile as tile
from concourse import bacc
from concourse.bass_utils import run_bass_kernel_spmd

F32 = mybir.dt.float32
F32R = mybir.dt.float32r
BF16 = mybir.dt.bfloat16
AF = mybir.ActivationFunctionType

B, S, D = 2, 4096, 1024
HEADS, DH, SEG, PM = 16, 64, 512, 16
INNER = HEADS * DH          # 1024
NCORES = 8
TOK = (B * S) // NCORES     # 1024 tokens per core
NSEG = TOK // SEG           # 2 segments per core
DT = D // 128               # 8 d tiles
NI2 = INNER // 128          # 8 inner tiles
EPS = 1e-6

# shifted attention blocks: (kcat col, q col start, n cols, psum col)
# kcat = [pm(16) | zeros(16) | k(512)]; block b covers kcat cols
# [128b, 128b+128), i.e. k rows [128b-32, 128b+96); b4 is the 32-row tail.
# Shift is 32 (not 16) so every partition access stays 32-aligned.
SH = 32
BLOCKS = [
    (0, 0, 512, 0),
    (128, 96, 416, 512),
    (256, 224, 288, 1024),
    (384, 352, 160, 1312),
    (512, 480, 32, 1472),
]
SIMW = 1536                 # sim psum tile cols (3 banks)
SIMUSED = 1504              # last written col (1472+32)


def build_bass(repeat=1):
    nc = bacc.Bacc("TRN2", target_bir_lowering=False, debug=False)

    xt_d = nc.dram_tensor("xt", [128, DT, TOK], BF16, kind="ExternalInput")
    wqk_d = nc.dram_tensor("w_qk", [2 * NI2, 128, DT, 128], BF16, kind="ExternalInput")
    wv_d = nc.dram_tensor("w_v", [128, DT, INNER], BF16, kind="ExternalInput")
    wo_d = nc.dram_tensor("w_out", [128, NI2, D], BF16, kind="ExternalInput")
    pmk_d = nc.dram_tensor("pm_kt", [128, NI2, PM], BF16, kind="ExternalInput")
    pmv_d = nc.dram_tensor("pm_vo", [PM, HEADS, DH + 1], BF16, kind="ExternalInput")
    maskf_d = nc.dram_tensor("maskf", [128, SIMW], BF16, kind="ExternalInput")
    ident_d = nc.dram_tensor("ident", [128, 128], BF16, kind="ExternalInput")
    hmask_d = nc.dram_tensor("hmask", [PM, NI2, 128], F32R, kind="ExternalInput")
    onesc_d = nc.dram_tensor("onesc", [128, 1], BF16, kind="ExternalInput")
    onesr_d = nc.dram_tensor("onesr", [1, 128], F32R, kind="ExternalInput")
    o_d = nc.dram_tensor("o", [TOK, D], F32, kind="ExternalOutput")

    with tile.TileContext(nc) as tc:
     for _rep in range(repeat):
      with ExitStack() as octx:
        consts = octx.enter_context(tc.tile_pool(name="consts", bufs=1))
        persist = octx.enter_context(tc.tile_pool(name="persist", bufs=1))

        maskf_sb = consts.tile([128, SIMW], BF16)
        nc.sync.dma_start(maskf_sb[:], maskf_d[:])
        ident_sb = consts.tile([128, 128], BF16)
        nc.sync.dma_start(ident_sb[:], ident_d[:])
        pmk_sb = consts.tile([128, NI2, PM], BF16)
        nc.sync.dma_start(pmk_sb[:], pmk_d[:])
        hmask_sb = consts.tile([PM, NI2, 128], F32R)
        nc.sync.dma_start(hmask_sb[:], hmask_d[:])
        onesc_sb = consts.tile([128, 1], BF16)
        nc.sync.dma_start(onesc_sb[:], onesc_d[:])
        onesr_sb = consts.tile([1, 128], F32R)
        nc.sync.dma_start(onesr_sb[:], onesr_d[:])
        eps_sb = consts.tile([1, 1], F32)
        nc.vector.memset(eps_sb[:], EPS)

        # activations / attention operands (x stays unnormalized; the rmsnorm
        # scale rs is folded into the q/k/v projection copies)
        xtall = persist.tile([128, DT, TOK], BF16)
        rsb = persist.tile([128, TOK], BF16)      # rs broadcast along partitions
        rsT = persist.tile([128, NSEG * 4], F32)   # rs as per-token columns
        qT = persist.tile([128, NI2, TOK], BF16)
        kcat = persist.tile([128, NI2, NSEG, SH + SEG], BF16)
        vcat = persist.tile([128, NSEG, HEADS, 5, DH + 1], BF16)
        aoT = persist.tile([128, NI2, TOK], BF16)

        # kcat header: pm_k in cols 0:16, zeros in the 16:32 pad; vcat block 0
        # rows 0:32 zeroed (pm_v lands in 0:16), ones column everywhere
        for seg in range(NSEG):
            nc.gpsimd.tensor_copy(kcat[:, :, seg, 0:PM], pmk_sb[:])
            nc.gpsimd.memset(kcat[:, :, seg, PM:SH], 0.0)
            nc.vector.memset(vcat[:, seg, :, :, DH : DH + 1], 1.0)
            # block-0 rows 0:32 fully zeroed INCLUDING the ones column: the
            # 16:32 pad rows have sim=0 -> p=1 beyond the masked 128 cols, so
            # they must contribute nothing to values OR denominators. pm_vo
            # (with its own ones column) then restores rows 0:16.
            nc.vector.memset(vcat[0:SH, seg, :, 0, :], 0.0)
            nc.sync.dma_start(vcat[0:PM, seg, :, 0, :], pmv_d[:])

        # resident weights: q/k projections first (B0 consumes them ot-by-ot
        # right after phase A0), then w_v (phase C0), w_out DMA'd later
        wqk_sb = persist.tile([128, 2 * NI2, DT, 128], BF16)
        wv_sb = persist.tile([128, DT, INNER], BF16)
        wo_sb = persist.tile([128, NI2, D], BF16)

        with ExitStack() as actx:
            sq_pool = actx.enter_context(tc.tile_pool(name="sq", bufs=1))
            st_pool = actx.enter_context(tc.tile_pool(name="st", bufs=2))
            psA = actx.enter_context(tc.tile_pool(name="psA", bufs=1, space="PSUM"))
            pre_ps = actx.enter_context(
                tc.tile_pool(name="pre_ps", bufs=3, space="PSUM")
            )

            def phase_a(seg):
                # rmsnorm statistics only: rs broadcast row (rsb) + per-token
                # columns (rsT); x itself stays raw, rs is applied in the
                # projection copies
                s0 = seg * SEG
                nc.sync.dma_start(
                    xtall[:, :, s0 : s0 + SEG], xt_d[:, :, s0 : s0 + SEG]
                )
                sq_t = sq_pool.tile([128, DT, SEG], BF16, tag="sq")
                nc.vector.tensor_mul(
                    sq_t[:], xtall[:, :, s0 : s0 + SEG], xtall[:, :, s0 : s0 + SEG]
                )
                ss = psA.tile([1, SEG], F32, tag="ss")
                for db in range(DT):
                    nc.tensor.matmul(
                        ss[:], onesc_sb[:], sq_t[:, db, :],
                        start=(db == 0), stop=(db == DT - 1),
                    )
                lnv = st_pool.tile([1, SEG], F32, tag="lnv")
                nc.scalar.activation(lnv[:], ss[:], AF.Ln, bias=eps_sb[:], scale=1.0 / D)
                rs = st_pool.tile([1, SEG], F32R, tag="rs")
                nc.scalar.activation(rs[:], lnv[:], AF.Exp, scale=-0.5)
                rb = psA.tile([128, SEG], F32, tag="rb")
                nc.tensor.matmul(rb[:], onesr_sb[:], rs[:], start=True, stop=True)
                nc.vector.tensor_copy(rsb[:, s0 : s0 + SEG], rb[:])
                for k4 in range(4):
                    tr = psA.tile([128, 128], BF16, tag="tr")
                    nc.tensor.transpose(
                        tr[:],
                        rsb[:, s0 + k4 * 128 : s0 + (k4 + 1) * 128],
                        ident_sb[:],
                    )
                    nc.vector.tensor_copy(
                        rsT[:, seg * 4 + k4 : seg * 4 + k4 + 1], tr[:, 0:1]
                    )

            def proj_chain(seg, ot, pool):
                # one q/k projection chain: 8 matmuls + rs-scaling copy out
                s0 = seg * SEG
                ps = pool.tile([128, SEG], F32, tag="mm")
                for db in range(DT):
                    nc.tensor.matmul(
                        ps[:], wqk_sb[:, ot, db, :], xtall[:, db, s0 : s0 + SEG],
                        start=(db == 0), stop=(db == DT - 1),
                    )
                if ot < NI2:
                    dst = qT[:, ot, s0 : s0 + SEG]
                else:
                    dst = kcat[:, ot - NI2, seg, SH : SH + SEG]
                nc.vector.tensor_mul(dst, ps[:], rsb[:, s0 : s0 + SEG])

            def v_chain(seg, k4, och, pool):
                # one v projection chain -> shifted vcat blocks, rs-scaled
                s0 = seg * SEG
                ps = pool.tile([128, SEG], F32, tag="mm")
                for db in range(DT):
                    nc.tensor.matmul(
                        ps[:],
                        xtall[:, db, s0 + k4 * 128 : s0 + (k4 + 1) * 128],
                        wv_sb[:, db, och * SEG : (och + 1) * SEG],
                        start=(db == 0), stop=(db == DT - 1),
                    )
                # partition-shifted writes: 32-partition pieces keep both the
                # PSUM source and the SBUF dest 32-aligned
                hs = slice(och * NI2, (och + 1) * NI2)
                rcol = seg * 4 + k4
                for q4 in range(4):
                    dst = (q4 + 1) * 32
                    blk = k4 if dst < 128 else k4 + 1
                    nc.vector.tensor_scalar_mul(
                        vcat[dst % 128 : dst % 128 + 32, seg, hs, blk, 0:DH],
                        ps[q4 * 32 : (q4 + 1) * 32, :].rearrange(
                            "p (h o) -> p h o", o=DH
                        ),
                        rsT[q4 * 32 : (q4 + 1) * 32, rcol : rcol + 1],
                    )

            def out_chain(seg, tt4, ech, pool, opool):
                s0 = seg * SEG
                ps = pool.tile([128, SEG], F32, tag="mm")
                for ti2 in range(NI2):
                    nc.tensor.matmul(
                        ps[:],
                        aoT[:, ti2, s0 + tt4 * 128 : s0 + (tt4 + 1) * 128],
                        wo_sb[:, ti2, ech * SEG : (ech + 1) * SEG],
                        start=(ti2 == 0), stop=(ti2 == NI2 - 1),
                    )
                osb = opool.tile([128, SEG], F32, tag="osb")
                nc.vector.tensor_copy(osb[:], ps[:])
                nc.sync.dma_start(
                    o_d[s0 + tt4 * 128 : s0 + (tt4 + 1) * 128,
                        ech * SEG : (ech + 1) * SEG],
                    osb[:],
                )

            # ---- serial head: A0 first (its x DMA leads the queue), then the
            # weight DMAs stream in B0/C0 consumption order
            phase_a(0)
            for ot in range(2 * NI2):
                nc.sync.dma_start(wqk_sb[:, ot], wqk_d[ot])
            nc.sync.dma_start(wv_sb[:], wv_d[:])
            for ot in range(2 * NI2):
                proj_chain(0, ot, pre_ps)
            for k4 in range(4):
                for och in range(2):
                    v_chain(0, k4, och, pre_ps)
            phase_a(1)
            nc.sync.dma_start(wo_sb[:], wo_d[:])

        # ---- attention (both segs) with interleaved projection fillers
        o_pool = octx.enter_context(tc.tile_pool(name="o", bufs=3))
        with ExitStack() as dctx:
            ps_sim = dctx.enter_context(
                tc.tile_pool(name="ps_sim", bufs=2, space="PSUM")
            )
            ps_pv = dctx.enter_context(tc.tile_pool(name="ps_pv", bufs=1, space="PSUM"))
            ps_proj = dctx.enter_context(
                tc.tile_pool(name="ps_proj", bufs=1, space="PSUM")
            )
            p_pool = dctx.enter_context(tc.tile_pool(name="p", bufs=3))
            den_pool = dctx.enter_context(tc.tile_pool(name="den", bufs=1))
            stage_pool = dctx.enter_context(tc.tile_pool(name="stage", bufs=2))

            def attn_seg(seg, fillers):
                s0 = seg * SEG
                den_seg = den_pool.tile([PM, SEG], F32, tag="den")
                nfill = len(fillers)
                fdone = 0
                for hd in range(HEADS):
                    pb = (hd % 2) * 64
                    ot = hd // 2
                    q_ap = qT[pb : pb + 64, ot, s0 : s0 + SEG]

                    sim = ps_sim.tile([128, SIMW], F32, tag="sim")
                    for kc, qs, n, pc in BLOCKS:
                        kw = 128 if kc < SEG else SH
                        nc.tensor.matmul(
                            sim[0:kw, pc : pc + n],
                            kcat[pb : pb + 64, ot, seg, kc : kc + kw],
                            q_ap[:, qs : qs + n],
                            start=True, stop=True,
                        )
                    p = p_pool.tile([128, SIMW], BF16, tag="p")
                    nc.scalar.activation(p[:, 0:SIMUSED], sim[:, 0:SIMUSED], AF.Exp)
                    # all causal/pm/pad masking in one full-width multiply
                    nc.gpsimd.tensor_mul(
                        p[:, 0:SIMUSED], p[:, 0:SIMUSED], maskf_sb[:, 0:SIMUSED]
                    )

                    pv = ps_pv.tile([DH + 1, SEG], F32, tag="pv")
                    for i, (kc, qs, n, pc) in enumerate(BLOCKS):
                        kw = 128 if i < 4 else SH
                        nc.tensor.matmul(
                            pv[:, qs : qs + n],
                            vcat[0:kw, seg, hd, i, :],
                            p[0:kw, pc : pc + n],
                            start=(i == 0), stop=(i == 4),
                        )
                    nc.vector.tensor_copy(
                        aoT[pb : pb + 64, ot, s0 : s0 + SEG], pv[0:DH, :]
                    )
                    dstage = stage_pool.tile([DH + 1, SEG], F32, tag="dst")
                    nc.vector.tensor_copy(dstage[DH : DH + 1, :], pv[DH : DH + 1, :])
                    nc.sync.dma_start(den_seg[hd : hd + 1, :], dstage[DH : DH + 1, :])

                    # emit interleaved filler chains (keeps PE fed during exps)
                    want = (hd + 1) * nfill // HEADS
                    while fdone < want:
                        fillers[fdone]()
                        fdone += 1

                rec = den_pool.tile([PM, SEG], F32R, tag="rec")
                with nc.allow_low_precision(reason="f32r reciprocal feeds matmul"):
                    nc.vector.reciprocal(rec[:], den_seg[:])
                for ti2 in range(NI2):
                    rb2 = ps_proj.tile([128, SEG], F32, tag="mm")
                    nc.tensor.matmul(
                        rb2[:], hmask_sb[:, ti2, :], rec[:], start=True, stop=True
                    )
                    ao_ap = aoT[:, ti2, s0 : s0 + SEG]
                    nc.vector.tensor_mul(ao_ap, ao_ap, rb2[:])

            fill0 = [
                (lambda ot=ot: proj_chain(1, ot, ps_proj))
                for ot in range(2 * NI2)
            ] + [
                (lambda k4=k4, och=och: v_chain(1, k4, och, ps_proj))
                for k4 in range(4)
                for och in range(2)
            ]
            attn_seg(0, fill0)
            fill1 = [
                (lambda tt4=tt4, ech=ech: out_chain(0, tt4, ech, ps_proj, o_pool))
                for tt4 in range(4)
                for ech in range(2)
            ]
            attn_seg(1, fill1)

        # ---- seg1 out-projection (post-attention tail, 3 PSUM banks)
        with ExitStack() as ectx:
            ps_o = ectx.enter_context(tc.tile_pool(name="ps_o", bufs=3, space="PSUM"))
            for tt4 in range(4):
                for ech in range(2):
                    out_chain(1, tt4, ech, ps_o, o_pool)

    nc.compile()
    return nc


_NC_CACHE = None


def _get_nc():
    global _NC_CACHE
    if _NC_CACHE is None:
        _NC_CACHE = build_bass()
    return _NC_CACHE


class _Runner:
    """Compile the Bass program once into a sharded jitted callable over the
    8 NeuronCores; reuse it for every kernel() invocation."""

    def __init__(self, nc):
        import jax
        from jax.sharding import Mesh, PartitionSpec
        from jax.experimental.shard_map import shard_map
        from concourse import bass2jax

        bass2jax.install_neuronx_cc_hook()
        self.nc = nc
        pname = nc.partition_id_tensor.name if nc.partition_id_tensor else None
        in_names, out_names, out_avals, self.zero_shapes = [], [], [], []
        for alloc in nc.m.functions[0].allocations:
            if not isinstance(alloc, mybir.MemoryLocationSet):
                continue
            name = alloc.memorylocations[0].name
            if alloc.kind == "ExternalInput":
                if name != pname:
                    in_names.append(name)
            elif alloc.kind == "ExternalOutput":
                out_names.append(name)
                shape = tuple(alloc.tensor_shape)
                dtype = mybir.dt.np(alloc.dtype)
                out_avals.append(jax.core.ShapedArray(shape, dtype))
                self.zero_shapes.append((shape, dtype))
        self.in_names, self.out_names = in_names, out_names
        all_in = in_names + out_names + ([pname] if pname else [])

        def _body(*args):
            operands = list(args)
            if pname is not None:
                operands.append(bass2jax.partition_id_tensor())
            return tuple(
                bass2jax._bass_exec_p.bind(
                    *operands,
                    out_avals=tuple(out_avals),
                    in_names=tuple(all_in),
                    out_names=tuple(out_names),
                    lowering_input_output_aliases=(),
                    sim_require_finite=False,
                    sim_require_nnan=False,
                    nc=nc,
                )
            )

        devices = jax.devices()[:NCORES]
        self.mesh = Mesh(np.asarray(devices), ("core",))
        self.sharding = jax.sharding.NamedSharding(self.mesh, PartitionSpec("core"))
        n_params = len(in_names)
        donate = tuple(range(n_params, n_params + len(out_names)))
        self.sharded = jax.jit(
            shard_map(
                _body,
                mesh=self.mesh,
                in_specs=(PartitionSpec("core"),) * (n_params + len(out_names)),
                out_specs=(PartitionSpec("core"),) * len(out_names),
                check_rep=False,
            ),
            donate_argnums=donate,
            keep_unused=True,
        )
        self._jax = jax

    def device_inputs(self, in_maps):
        concat = [
            np.concatenate([np.asarray(m[nm]) for m in in_maps], axis=0)
            for nm in self.in_names
        ]
        return [self._jax.device_put(a, self.sharding) for a in concat]

    def zeros(self):
        return [
            self._jax.device_put(
                np.zeros((NCORES * s[0], *s[1:]), d), self.sharding
            )
            for s, d in self.zero_shapes
        ]

    def __call__(self, dev_in):
        outs = self.sharded(*dev_in, *self.zeros())
        for o in outs:
            o.block_until_ready()
        return outs


_RUNNER = None


def _get_runner():
    global _RUNNER
    if _RUNNER is None:
        _RUNNER = _Runner(_get_nc())
    return _RUNNER


def make_in_maps(x, gamma, w_qkv, w_out, pm_k, pm_v):
    bf = ml_dtypes.bfloat16
    x = np.asarray(x, dtype=np.float32).reshape(B * S, D)
    gamma = np.asarray(gamma, dtype=np.float32)
    w_qkv = np.asarray(w_qkv, dtype=np.float32)
    w_out = np.asarray(w_out, dtype=np.float32)
    pm_k = np.asarray(pm_k, dtype=np.float32)
    pm_v = np.asarray(pm_v, dtype=np.float32)

    w = w_qkv * gamma[:, None]
    scale = DH ** -0.5
    wqk = np.concatenate([w[:, :INNER] * scale, w[:, INNER : 2 * INNER]], axis=1)
    # [D, 2*INNER] -> [ot, p, db, c]
    wqk_t = np.ascontiguousarray(
        wqk.reshape(DT, 128, 2 * NI2, 128).transpose(2, 1, 0, 3)
    ).astype(bf)
    wv_t = np.ascontiguousarray(
        w[:, 2 * INNER :].reshape(DT, 128, INNER).transpose(1, 0, 2)
    ).astype(bf)
    wo_t = np.ascontiguousarray(
        w_out.reshape(NI2, 128, D).transpose(1, 0, 2)
    ).astype(bf)

    pmk_t = np.zeros((128, NI2, PM), dtype=np.float32)
    for hd in range(HEADS):
        pmk_t[(hd % 2) * 64 : (hd % 2) * 64 + 64, hd // 2, :] = pm_k[hd].T
    pmk_t = pmk_t.astype(bf)
    pmv_o = np.zeros((PM, HEADS, DH + 1), dtype=np.float32)
    pmv_o[:, :, :DH] = pm_v.transpose(1, 0, 2)
    pmv_o[:, :, DH] = 1.0
    pmv_o = pmv_o.astype(bf)

    # full-width attention mask over the packed sim layout [128, SIMW]
    p_ = np.arange(128)[:, None]
    maskf = np.zeros((128, SIMW), dtype=np.float32)
    c0 = np.arange(512)[None, :]
    maskf[:, 0:512] = np.where(
        p_ < PM, 1.0, np.where(p_ < SH, 0.0, c0 >= p_ - SH)
    )
    for _, qs, n, pc in BLOCKS[1:]:
        cb = np.arange(n)[None, :]
        maskf[:, pc : pc + n] = cb >= p_
    maskf[SH:, 1472:1504] = 0.0          # b4 tail: only 32 valid rows
    maskf = maskf.astype(bf)
    ident = np.eye(128, dtype=bf)
    hmask = np.zeros((PM, NI2, 128), dtype=np.float32)
    for ti2 in range(NI2):
        for m in range(128):
            hmask[(ti2 * 128 + m) // DH, ti2, m] = 1.0
    onesc = np.ones((128, 1), dtype=bf)
    onesr = np.ones((1, 128), dtype=np.float32)

    shared = {
        "w_qk": wqk_t,
        "w_v": wv_t,
        "w_out": wo_t,
        "pm_kt": pmk_t,
        "pm_vo": pmv_o,
        "maskf": maskf,
        "ident": ident,
        "hmask": hmask,
        "onesc": onesc,
        "onesr": onesr,
    }
    maps = []
    for c in range(NCORES):
        xc = x[c * TOK : (c + 1) * TOK].T.reshape(DT, 128, TOK).transpose(1, 0, 2)
        maps.append({"xt": np.ascontiguousarray(xc).astype(bf), **shared})
    return maps


def kernel(x, gamma, w_qkv, w_out, pm_k, pm_v):
    runner = _get_runner()
    in_maps = make_in_maps(x, gamma, w_qkv, w_out, pm_k, pm_v)
    outs = runner(runner.device_inputs(in_maps))
    out = np.asarray(outs[0])          # [NCORES*TOK, D] global row-sharded
    return out.reshape(B, S, D)


if __name__ == "__main__":
    rng = np.random.default_rng(0)
    ins = {
        "x": rng.standard_normal((B, S, D), dtype=np.float32),
        "gamma": np.ones(D, dtype=np.float32),
        "w_qkv": (rng.standard_normal((D, 3 * INNER), dtype=np.float32) * D**-0.5),
        "w_out": (rng.standard_normal((INNER, D), dtype=np.float32) * INNER**-0.5),
        "pm_k": (rng.standard_normal((HEADS, PM, DH), dtype=np.float32) * 0.02),
        "pm_v": (rng.standard_normal((HEADS, PM, DH), dtype=np.float32) * 0.02),
    }
    out = kernel(**ins)
    print("out", out.shape, out.dtype, np.abs(out).mean())


# revision 34
# speedup vs baseline: 28.0441x; 28.0441x over previous
"""Trainium2 Bass kernel for MemoryAsContextTransformer segmented attention.

Reference computation (per full input):
  h   = rmsnorm(x, gamma)                      [B=2, S=4096, D=1024]
  qkv = h @ w_qkv                              heads=16, dh=64, seg=512, pm=16
  per (batch, segment, head): block-causal attention with 16 persistent
  memory tokens prepended to k/v, softmax, out = attn @ v
  out @ w_out                                  [2, 4096, 1024]

Sharding: data-parallel over the 16 (batch, segment) units; 2 contiguous
segments (1024 tokens) per core, full weights broadcast to all 8 cores.

Kernel design (all matmul operands bf16, f32 PSUM accumulation):
  x arrives host-pre-transposed as xT [d, t]; rmsnorm computed via
  PE partition-reduction (ones-column matmul of x^2) + Act ln/exp for
  rsqrt + PE one-hot broadcast; h = xT * rs on DVE. No PE transposes.
  Projections: stationary weight tiles resident in SBUF, 8-matmul
  accumulation chains at N=512.
  Attention uses a shifted-block scheme: k/v get the 16 persistent-memory
  rows prepended, blocks of 128 k-rows start at -16 so the pm rows ride in
  block 0; per (head, seg): 5 sim matmuls into one 3-bank PSUM tile, ONE
  exp over it (Act), 5 causal-mask muls (GpSimd), 5 PV matmuls with an
  extra ones column for the softmax denominators.
  Normalization: denominator rows gathered per segment via tiny SBUF DMAs,
  reciprocal (DVE), one-hot head-broadcast matmul, DVE muls.
  Out-projection chains per segment; seg0's runs interleaved into seg1's
  attention, and seg1's projections interleave into seg0's attention, so
  the PE queue always has independent work while Act runs the exps.
"""

import sys

sys.path.insert(0, "/opt/trn_rl_repo")

from contextlib import ExitStack

import numpy as np
import ml_dtypes

import concourse.bass as bass
import concourse.mybir as mybir
import concourse.tile as tile
from concourse import bacc
from concourse.bass_utils import run_bass_kernel_spmd

F32 = mybir.dt.float32
F32R = mybir.dt.float32r
BF16 = mybir.dt.bfloat16
AF = mybir.ActivationFunctionType

B, S, D = 2, 4096, 1024
HEADS, DH, SEG, PM = 16, 64, 512, 16
INNER = HEADS * DH          # 1024
NCORES = 8
TOK = (B * S) // NCORES     # 1024 tokens per core
NSEG = TOK // SEG           # 2 segments per core
DT = D // 128               # 8 d tiles
NI2 = INNER // 128          # 8 inner tiles
EPS = 1e-6

# shifted attention blocks: (kcat col, q col start, n cols, psum col)
# kcat = [pm(16) | zeros(16) | k(512)]; block b covers kcat cols
# [128b, 128b+128), i.e. k rows [128b-32, 128b+96); b4 is the 32-row tail.
# Shift is 32 (not 16) so every partition access stays 32-aligned.
SH = 32
BLOCKS = [
    (0, 0, 512, 0),
    (128, 96, 416, 512),
    (256, 224, 288, 1024),
    (384, 352, 160, 1312),
    (512, 480, 32, 1472),
]
SIMW = 1536                 # sim psum tile cols (3 banks)
SIMUSED = 1504              # last written col (1472+32)


def build_bass(repeat=1):
    nc = bacc.Bacc("TRN2", target_bir_lowering=False, debug=False)

    xt_d = nc.dram_tensor("xt", [128, DT, TOK], BF16, kind="ExternalInput")
    wqk_d = nc.dram_tensor("w_qk", [2 * NI2, 128, DT, 128], BF16, kind="ExternalInput")
    wv_d = nc.dram_tensor("w_v", [128, DT, INNER], BF16, kind="ExternalInput")
    wo_d = nc.dram_tensor("w_out", [128, NI2, D], BF16, kind="ExternalInput")
    pmk_d = nc.dram_tensor("pm_kt", [128, NI2, PM], BF16, kind="ExternalInput")
    pmv_d = nc.dram_tensor("pm_vo", [PM, HEADS, DH + 1], BF16, kind="ExternalInput")
    maskf_d = nc.dram_tensor("maskf", [128, SIMW], BF16, kind="ExternalInput")
    ident_d = nc.dram_tensor("ident", [128, 128], BF16, kind="ExternalInput")
    hmask_d = nc.dram_tensor("hmask", [PM, NI2, 128], F32R, kind="ExternalInput")
    onesc_d = nc.dram_tensor("onesc", [128, 1], BF16, kind="ExternalInput")
    onesr_d = nc.dram_tensor("onesr", [1, 128], F32R, kind="ExternalInput")
    o_d = nc.dram_tensor("o", [TOK, D], F32, kind="ExternalOutput")

    with tile.TileContext(nc) as tc:
     for _rep in range(repeat):
      with ExitStack() as octx:
        consts = octx.enter_context(tc.tile_pool(name="consts", bufs=1))
        persist = octx.enter_context(tc.tile_pool(name="persist", bufs=1))

        maskf_sb = consts.tile([128, SIMW], BF16)
        nc.sync.dma_start(maskf_sb[:], maskf_d[:])
        ident_sb = consts.tile([128, 128], BF16)
        nc.sync.dma_start(ident_sb[:], ident_d[:])
        pmk_sb = consts.tile([128, NI2, PM], BF16)
        nc.sync.dma_start(pmk_sb[:], pmk_d[:])
        hmask_sb = consts.tile([PM, NI2, 128], F32R)
        nc.sync.dma_start(hmask_sb[:], hmask_d[:])
        onesc_sb = consts.tile([128, 1], BF16)
        nc.sync.dma_start(onesc_sb[:], onesc_d[:])
        onesr_sb = consts.tile([1, 128], F32R)
        nc.sync.dma_start(onesr_sb[:], onesr_d[:])
        eps_sb = consts.tile([1, 1], F32)
        nc.vector.memset(eps_sb[:], EPS)
        # pre-warm the Act function table (ln+exp set) under the initial DMAs
        warm_sb = consts.tile([1, 1], F32)
        nc.scalar.activation(warm_sb[:], eps_sb[:], AF.Ln)
        nc.scalar.activation(warm_sb[:], eps_sb[:], AF.Exp)

        # activations / attention operands (x stays unnormalized; the rmsnorm
        # scale rs is folded into the q/k/v projection copies)
        xtall = persist.tile([128, DT, TOK], BF16)
        rsb = persist.tile([128, TOK], BF16)       # rs broadcast along partitions
        rsT = persist.tile([128, NSEG, 5], F32)    # rs per-token cols per v-block
        qT = persist.tile([128, NI2, TOK], BF16)
        kcat = persist.tile([128, NI2, NSEG, SH + SEG], BF16)
        vcat = persist.tile([128, NSEG, HEADS, 5, DH + 1], BF16)
        aoT = persist.tile([128, NI2, TOK], BF16)

        # kcat header: pm_k in cols 0:16, zeros in the 16:32 pad; vcat block 0
        # rows 0:32 zeroed (pm_v lands in 0:16), ones column everywhere
        for seg in range(NSEG):
            nc.gpsimd.tensor_copy(kcat[:, :, seg, 0:PM], pmk_sb[:])
            nc.gpsimd.memset(kcat[:, :, seg, PM:SH], 0.0)
            nc.vector.memset(vcat[:, seg, :, :, DH : DH + 1], 1.0)
            # block-0 rows 0:32 fully zeroed INCLUDING the ones column: the
            # 16:32 pad rows have sim=0 -> p=1 beyond the masked 128 cols, so
            # they must contribute nothing to values OR denominators. pm_vo
            # (with its own ones column) then restores rows 0:16.
            nc.vector.memset(vcat[0:SH, seg, :, 0, :], 0.0)
            nc.sync.dma_start(vcat[0:PM, seg, :, 0, :], pmv_d[:])

        # resident weights: q/k projections first (B0 consumes them ot-by-ot
        # right after phase A0), then w_v (phase C0), w_out DMA'd later
        wqk_sb = persist.tile([128, 2 * NI2, DT, 128], BF16)
        wv_sb = persist.tile([128, DT, INNER], BF16)
        wo_sb = persist.tile([128, NI2, D], BF16)

        with ExitStack() as actx:
            sq_pool = actx.enter_context(tc.tile_pool(name="sq", bufs=1))
            st_pool = actx.enter_context(tc.tile_pool(name="st", bufs=2))
            psA = actx.enter_context(tc.tile_pool(name="psA", bufs=1, space="PSUM"))
            pre_ps = actx.enter_context(
                tc.tile_pool(name="pre_ps", bufs=3, space="PSUM")
            )

            def phase_a(seg):
                # rmsnorm statistics only: rs broadcast row (rsb) + per-token
                # columns (rsT); x itself stays raw, rs is applied in the
                # projection copies
                s0 = seg * SEG
                nc.sync.dma_start(
                    xtall[:, :, s0 : s0 + SEG], xt_d[:, :, s0 : s0 + SEG]
                )
                sq_t = sq_pool.tile([128, DT, SEG], BF16, tag="sq")
                nc.vector.tensor_mul(
                    sq_t[:], xtall[:, :, s0 : s0 + SEG], xtall[:, :, s0 : s0 + SEG]
                )
                ss = psA.tile([1, SEG], F32, tag="ss")
                for db in range(DT):
                    nc.tensor.matmul(
                        ss[:], onesc_sb[:], sq_t[:, db, :],
                        start=(db == 0), stop=(db == DT - 1),
                    )
                lnv = st_pool.tile([1, SEG], F32, tag="lnv")
                nc.scalar.activation(lnv[:], ss[:], AF.Ln, bias=eps_sb[:], scale=1.0 / D)
                rs = st_pool.tile([1, SEG], F32R, tag="rs")
                nc.scalar.activation(rs[:], lnv[:], AF.Exp, scale=-0.5)
                rb = psA.tile([128, SEG], F32, tag="rb")
                nc.tensor.matmul(rb[:], onesr_sb[:], rs[:], start=True, stop=True)
                nc.vector.tensor_copy(rsb[:, s0 : s0 + SEG], rb[:])
                # rs as per-token columns, one per v-block: blk 0 unshifted
                # (consumed by the 32-partition split copies), blks 1..4 at
                # token offset 128*blk-SH matching their vcat partition layout
                for blk in range(5):
                    lo = 0 if blk == 0 else 128 * blk - SH
                    w = 32 if blk == 4 else 128
                    tr = psA.tile([128, 128], BF16, tag="tr")
                    nc.tensor.transpose(
                        tr[0:w, :],
                        rsb[:, s0 + lo : s0 + lo + w],
                        ident_sb[:],
                    )
                    nc.vector.tensor_copy(
                        rsT[0:w, seg, blk : blk + 1], tr[0:w, 0:1]
                    )

            def proj_chain(seg, ot, pool):
                # one q/k projection chain: 8 matmuls + rs-scaling copy out
                s0 = seg * SEG
                ps = pool.tile([128, SEG], F32, tag="mm")
                for db in range(DT):
                    nc.tensor.matmul(
                        ps[:], wqk_sb[:, ot, db, :], xtall[:, db, s0 : s0 + SEG],
                        start=(db == 0), stop=(db == DT - 1),
                    )
                if ot < NI2:
                    dst = qT[:, ot, s0 : s0 + SEG]
                else:
                    dst = kcat[:, ot - NI2, seg, SH : SH + SEG]
                nc.vector.tensor_mul(dst, ps[:], rsb[:, s0 : s0 + SEG])

            def v_chain(seg, blk, och, pool):
                # one v projection chain writing one vcat block, rs-scaled.
                # The stationary x slice is token-shifted by -SH so the PE
                # output partitions line up with the block's shifted layout;
                # block 0 (whose first SH partitions are pm/pad) is produced
                # 0-based and copied in three 32-partition pieces instead.
                s0 = seg * SEG
                if blk == 0:
                    lo, m = 0, 128 - SH
                elif blk == 4:
                    lo, m = 128 * blk - SH, SH
                else:
                    lo, m = 128 * blk - SH, 128
                ps = pool.tile([128, SEG], F32, tag="mm")
                for db in range(DT):
                    nc.tensor.matmul(
                        ps[0:m, :],
                        xtall[:, db, s0 + lo : s0 + lo + m],
                        wv_sb[:, db, och * SEG : (och + 1) * SEG],
                        start=(db == 0), stop=(db == DT - 1),
                    )
                hs = slice(och * NI2, (och + 1) * NI2)
                if blk == 0:
                    for q4 in range(3):
                        nc.vector.tensor_scalar_mul(
                            vcat[SH + q4 * 32 : SH + (q4 + 1) * 32, seg, hs, 0, 0:DH],
                            ps[q4 * 32 : (q4 + 1) * 32, :].rearrange(
                                "p (h o) -> p h o", o=DH
                            ),
                            rsT[q4 * 32 : (q4 + 1) * 32, seg, 0:1],
                        )
                else:
                    nc.vector.tensor_scalar_mul(
                        vcat[0:m, seg, hs, blk, 0:DH],
                        ps[0:m, :].rearrange("p (h o) -> p h o", o=DH),
                        rsT[0:m, seg, blk : blk + 1],
                    )

            def out_chain(seg, tt4, ech, pool, opool):
                s0 = seg * SEG
                ps = pool.tile([128, SEG], F32, tag="mm")
                for ti2 in range(NI2):
                    nc.tensor.matmul(
                        ps[:],
                        aoT[:, ti2, s0 + tt4 * 128 : s0 + (tt4 + 1) * 128],
                        wo_sb[:, ti2, ech * SEG : (ech + 1) * SEG],
                        start=(ti2 == 0), stop=(ti2 == NI2 - 1),
                    )
                osb = opool.tile([128, SEG], F32, tag="osb")
                nc.vector.tensor_copy(osb[:], ps[:])
                nc.sync.dma_start(
                    o_d[s0 + tt4 * 128 : s0 + (tt4 + 1) * 128,
                        ech * SEG : (ech + 1) * SEG],
                    osb[:],
                )

            # ---- serial head: A0 first (its x DMA leads the queue), then the
            # weight DMAs stream in B0/C0 consumption order
            phase_a(0)
            for ot in range(2 * NI2):
                nc.sync.dma_start(wqk_sb[:, ot], wqk_d[ot])
            nc.sync.dma_start(wv_sb[:], wv_d[:])
            for ot in range(2 * NI2):
                proj_chain(0, ot, pre_ps)
            for blk in range(5):
                for och in range(2):
                    v_chain(0, blk, och, pre_ps)
            phase_a(1)
            nc.sync.dma_start(wo_sb[:], wo_d[:])

        # ---- attention (both segs) with interleaved projection fillers
        o_pool = octx.enter_context(tc.tile_pool(name="o", bufs=3))
        with ExitStack() as dctx:
            ps_sim = dctx.enter_context(
                tc.tile_pool(name="ps_sim", bufs=2, space="PSUM")
            )
            ps_pv = dctx.enter_context(tc.tile_pool(name="ps_pv", bufs=1, space="PSUM"))
            ps_proj = dctx.enter_context(
                tc.tile_pool(name="ps_proj", bufs=1, space="PSUM")
            )
            p_pool = dctx.enter_context(tc.tile_pool(name="p", bufs=3))
            den_pool = dctx.enter_context(tc.tile_pool(name="den", bufs=1))
            stage_pool = dctx.enter_context(tc.tile_pool(name="stage", bufs=2))

            def attn_seg(seg, fillers):
                s0 = seg * SEG
                den_seg = den_pool.tile([PM, SEG], F32, tag="den")
                nfill = len(fillers)
                fdone = 0
                for hd in range(HEADS):
                    pb = (hd % 2) * 64
                    ot = hd // 2
                    q_ap = qT[pb : pb + 64, ot, s0 : s0 + SEG]

                    sim = ps_sim.tile([128, SIMW], F32, tag="sim")
                    for kc, qs, n, pc in BLOCKS:
                        kw = 128 if kc < SEG else SH
                        nc.tensor.matmul(
                            sim[0:kw, pc : pc + n],
                            kcat[pb : pb + 64, ot, seg, kc : kc + kw],
                            q_ap[:, qs : qs + n],
                            start=True, stop=True,
                        )
                    p = p_pool.tile([128, SIMW], BF16, tag="p")
                    nc.scalar.activation(p[:, 0:SIMUSED], sim[:, 0:SIMUSED], AF.Exp)
                    # all causal/pm/pad masking in one full-width multiply
                    # (DVE: bf16 packed 2x mode; GpSimd measured ~4x slower)
                    nc.vector.tensor_mul(
                        p[:, 0:SIMUSED], p[:, 0:SIMUSED], maskf_sb[:, 0:SIMUSED]
                    )

                    pv = ps_pv.tile([DH + 1, SEG], F32, tag="pv")
                    for i, (kc, qs, n, pc) in enumerate(BLOCKS):
                        kw = 128 if i < 4 else SH
                        nc.tensor.matmul(
                            pv[:, qs : qs + n],
                            vcat[0:kw, seg, hd, i, :],
                            p[0:kw, pc : pc + n],
                            start=(i == 0), stop=(i == 4),
                        )
                    nc.vector.tensor_copy(
                        aoT[pb : pb + 64, ot, s0 : s0 + SEG], pv[0:DH, :]
                    )
                    # denominator row: stage at partition DH, DMA to head slot
                    dstage = stage_pool.tile([DH + 1, SEG], F32, tag="dst")
                    nc.vector.tensor_copy(dstage[DH : DH + 1, :], pv[DH : DH + 1, :])
                    nc.sync.dma_start(den_seg[hd : hd + 1, :], dstage[DH : DH + 1, :])

                    # emit interleaved filler chains (keeps PE fed during exps)
                    want = (hd + 1) * nfill // HEADS
                    while fdone < want:
                        fillers[fdone]()
                        fdone += 1

                rec_f = den_pool.tile([PM, SEG], F32, tag="recf")
                nc.vector.reciprocal_approx_fast(rec_f[:], den_seg[:])
                rec = den_pool.tile([PM, SEG], F32R, tag="rec")
                nc.vector.tensor_copy(rec[:], rec_f[:])
                for ti2 in range(NI2):
                    rb2 = ps_proj.tile([128, SEG], F32, tag="mm")
                    nc.tensor.matmul(
                        rb2[:], hmask_sb[:, ti2, :], rec[:], start=True, stop=True
                    )
                    ao_ap = aoT[:, ti2, s0 : s0 + SEG]
                    nc.vector.tensor_mul(ao_ap, ao_ap, rb2[:])

            fill0 = [
                (lambda ot=ot: proj_chain(1, ot, ps_proj))
                for ot in range(2 * NI2)
            ] + [
                (lambda blk=blk, och=och: v_chain(1, blk, och, ps_proj))
                for blk in range(5)
                for och in range(2)
            ]
            attn_seg(0, fill0)
            fill1 = [
                (lambda tt4=tt4, ech=ech: out_chain(0, tt4, ech, ps_proj, o_pool))
                for tt4 in range(4)
                for ech in range(2)
            ]
            attn_seg(1, fill1)

        # ---- seg1 out-projection (post-attention tail, 3 PSUM banks)
        with ExitStack() as ectx:
            ps_o = ectx.enter_context(tc.tile_pool(name="ps_o", bufs=3, space="PSUM"))
            for tt4 in range(4):
                for ech in range(2):
                    out_chain(1, tt4, ech, ps_o, o_pool)

    nc.compile()
    return nc


_NC_CACHE = None


def _get_nc():
    global _NC_CACHE
    if _NC_CACHE is None:
        _NC_CACHE = build_bass()
    return _NC_CACHE


class _Runner:
    """Compile the Bass program once into a sharded jitted callable over the
    8 NeuronCores; reuse it for every kernel() invocation."""

    def __init__(self, nc):
        import jax
        from jax.sharding import Mesh, PartitionSpec
        from jax.experimental.shard_map import shard_map
        from concourse import bass2jax

        bass2jax.install_neuronx_cc_hook()
        self.nc = nc
        pname = nc.partition_id_tensor.name if nc.partition_id_tensor else None
        in_names, out_names, out_avals, self.zero_shapes = [], [], [], []
        for alloc in nc.m.functions[0].allocations:
            if not isinstance(alloc, mybir.MemoryLocationSet):
                continue
            name = alloc.memorylocations[0].name
            if alloc.kind == "ExternalInput":
                if name != pname:
                    in_names.append(name)
            elif alloc.kind == "ExternalOutput":
                out_names.append(name)
                shape = tuple(alloc.tensor_shape)
                dtype = mybir.dt.np(alloc.dtype)
                out_avals.append(jax.core.ShapedArray(shape, dtype))
                self.zero_shapes.append((shape, dtype))
        self.in_names, self.out_names = in_names, out_names
        all_in = in_names + out_names + ([pname] if pname else [])

        def _body(*args):
            operands = list(args)
            if pname is not None:
                operands.append(bass2jax.partition_id_tensor())
            return tuple(
                bass2jax._bass_exec_p.bind(
                    *operands,
                    out_avals=tuple(out_avals),
                    in_names=tuple(all_in),
                    out_names=tuple(out_names),
                    lowering_input_output_aliases=(),
                    sim_require_finite=False,
                    sim_require_nnan=False,
                    nc=nc,
                )
            )

        devices = jax.devices()[:NCORES]
        self.mesh = Mesh(np.asarray(devices), ("core",))
        self.sharding = jax.sharding.NamedSharding(self.mesh, PartitionSpec("core"))
        n_params = len(in_names)
        donate = tuple(range(n_params, n_params + len(out_names)))
        self.sharded = jax.jit(
            shard_map(
                _body,
                mesh=self.mesh,
                in_specs=(PartitionSpec("core"),) * (n_params + len(out_names)),
                out_specs=(PartitionSpec("core"),) * len(out_names),
                check_rep=False,
            ),
            donate_argnums=donate,
            keep_unused=True,
        )
        self._jax = jax

    def device_inputs(self, in_maps):
        concat = [
            np.concatenate([np.asarray(m[nm]) for m in in_maps], axis=0)
            for nm in self.in_names
        ]
        return [self._jax.device_put(a, self.sharding) for a in concat]

    def zeros(self):
        return [
            self._jax.device_put(
                np.zeros((NCORES * s[0], *s[1:]), d), self.sharding
            )
            for s, d in self.zero_shapes
        ]

    def __call__(self, dev_in):
        outs = self.sharded(*dev_in, *self.zeros())
        for o in outs:
            o.block_until_ready()
        return outs


_RUNNER = None


def _get_runner():
    global _RUNNER
    if _RUNNER is None:
        _RUNNER = _Runner(_get_nc())
    return _RUNNER


def make_in_maps(x, gamma, w_qkv, w_out, pm_k, pm_v):
    bf = ml_dtypes.bfloat16
    x = np.asarray(x, dtype=np.float32).reshape(B * S, D)
    gamma = np.asarray(gamma, dtype=np.float32)
    w_qkv = np.asarray(w_qkv, dtype=np.float32)
    w_out = np.asarray(w_out, dtype=np.float32)
    pm_k = np.asarray(pm_k, dtype=np.float32)
    pm_v = np.asarray(pm_v, dtype=np.float32)

    w = w_qkv * gamma[:, None]
    scale = DH ** -0.5
    wqk = np.concatenate([w[:, :INNER] * scale, w[:, INNER : 2 * INNER]], axis=1)
    # [D, 2*INNER] -> [ot, p, db, c]
    wqk_t = np.ascontiguousarray(
        wqk.reshape(DT, 128, 2 * NI2, 128).transpose(2, 1, 0, 3)
    ).astype(bf)
    wv_t = np.ascontiguousarray(
        w[:, 2 * INNER :].reshape(DT, 128, INNER).transpose(1, 0, 2)
    ).astype(bf)
    wo_t = np.ascontiguousarray(
        w_out.reshape(NI2, 128, D).transpose(1, 0, 2)
    ).astype(bf)

    pmk_t = np.zeros((128, NI2, PM), dtype=np.float32)
    for hd in range(HEADS):
        pmk_t[(hd % 2) * 64 : (hd % 2) * 64 + 64, hd // 2, :] = pm_k[hd].T
    pmk_t = pmk_t.astype(bf)
    pmv_o = np.zeros((PM, HEADS, DH + 1), dtype=np.float32)
    pmv_o[:, :, :DH] = pm_v.transpose(1, 0, 2)
    pmv_o[:, :, DH] = 1.0
    pmv_o = pmv_o.astype(bf)

    # full-width attention mask over the packed sim layout [128, SIMW]
    p_ = np.arange(128)[:, None]
    maskf = np.zeros((128, SIMW), dtype=np.float32)
    c0 = np.arange(512)[None, :]
    maskf[:, 0:512] = np.where(
        p_ < PM, 1.0, np.where(p_ < SH, 0.0, c0 >= p_ - SH)
    )
    for _, qs, n, pc in BLOCKS[1:]:
        cb = np.arange(n)[None, :]
        maskf[:, pc : pc + n] = cb >= p_
    maskf[SH:, 1472:1504] = 0.0          # b4 tail: only 32 valid rows
    maskf = maskf.astype(bf)
    ident = np.eye(128, dtype=bf)
    hmask = np.zeros((PM, NI2, 128), dtype=np.float32)
    for ti2 in range(NI2):
        for m in range(128):
            hmask[(ti2 * 128 + m) // DH, ti2, m] = 1.0
    onesc = np.ones((128, 1), dtype=bf)
    onesr = np.ones((1, 128), dtype=np.float32)

    shared = {
        "w_qk": wqk_t,
        "w_v": wv_t,
        "w_out": wo_t,
        "pm_kt": pmk_t,
        "pm_vo": pmv_o,
        "maskf": maskf,
        "ident": ident,
        "hmask": hmask,
        "onesc": onesc,
        "onesr": onesr,
    }
    maps = []
    for c in range(NCORES):
        xc = x[c * TOK : (c + 1) * TOK].T.reshape(DT, 128, TOK).transpose(1, 0, 2)
        maps.append({"xt": np.ascontiguousarray(xc).astype(bf), **shared})
    return maps


def kernel(x, gamma, w_qkv, w_out, pm_k, pm_v):
    runner = _get_runner()
    in_maps = make_in_maps(x, gamma, w_qkv, w_out, pm_k, pm_v)
    outs = runner(runner.device_inputs(in_maps))
    out = np.asarray(outs[0])          # [NCORES*TOK, D] global row-sharded
    return out.reshape(B, S, D)


if __name__ == "__main__":
    rng = np.random.default_rng(0)
    ins = {
        "x": rng.standard_normal((B, S, D), dtype=np.float32),
        "gamma": np.ones(D, dtype=np.float32),
        "w_qkv": (rng.standard_normal((D, 3 * INNER), dtype=np.float32) * D**-0.5),
        "w_out": (rng.standard_normal((INNER, D), dtype=np.float32) * INNER**-0.5),
        "pm_k": (rng.standard_normal((HEADS, PM, DH), dtype=np.float32) * 0.02),
        "pm_v": (rng.standard_normal((HEADS, PM, DH), dtype=np.float32) * 0.02),
    }
    out = kernel(**ins)
    print("out", out.shape, out.dtype, np.abs(out).mean())
